# revision 7
# baseline (speedup 1.0000x reference)
"""Trainium2 Bass kernel for 4D valid convolution.

x (2,2,32,32,64,64) f32, weight (4,2,3,3,3,3) f32, bias (4,) f32
-> out (2,4,30,30,62,62) f32  (valid cross-correlation + bias)

Strategy: 8 cores = batch(2) x a-quadrant(4). Each core computes
out[b, :, a_sel, :, :, :] from slab x[b, :, a0:a0+10, :, :, :].

The d-dimension kernel taps use Winograd F(2,3): 4 transform points per
2 outputs replace the 3 direct taps, cutting TensorE columns 1.5x.
The host sends x with d deinterleaved into (even, odd) planes so every
transform op is a packed (stride-1) bf16 tensor_tensor:
       p0=xe[t]-xe[t+1]  p1=xo[t]+xe[t+1]  p2=xe[t+1]-xo[t]
       p3=xo[t]-xo[t+1]            (xe[t]=x[2t], xo[t]=x[2t+1])
TensorE per (a,b)-block and c-chunk: for each point p, 3 matmuls
(c-taps k) accumulate Y_p in its own PSUM bank:
       K (partitions) = (b-window=6, ci=2, a-window=10) = 120
       M (psum)       = (co=4, a_out=8, b_out=4) = 128
       N              = wc x 31 d-pairs <= 496 (1 PSUM bank)
       lhsT = host-prebuilt banded matrices of G-transformed weights.
Inverse transform (each op reads at most one PSUM operand, each Y bank
is read exactly once; bias folded in):
       Act:  s1=copy(Y1)  s2=copy(Y2)
       DVE:  u=s1+bias+Y0   v=s1+bias-Y3
       Pool: even=u+s2      odd=v-s2     (packed planes; host interleaves)

Startup: the first chunk's rhs columns (c<18) load first, split across
the Sync+ACT DGE queues; weights ride the GpSimd queue. Output goes to
DRAM partition-major per (b-block, c-chunk, parity); host unscrambles.
"""

import sys

if "/opt/trn_rl_repo" not in sys.path:
    sys.path.insert(0, "/opt/trn_rl_repo")

import ml_dtypes
import numpy as np

BF16 = ml_dtypes.bfloat16

B, CI, CO = 2, 2, 4
A, B2, C, D = 32, 32, 64, 64
AO, BO, CL, DL = 30, 30, 62, 62
K = 3
TD = 31  # d-pair tiles (Winograd F(2,3) stride 2)

# per-core a-slab starts; each core computes 8 output a-rows (q=3 overlaps q=2)
A0 = [0, 8, 16, 22]
SA = 10  # a-window (8 outputs + 2 halo)
SB = 6  # b-window per block (4 outputs + 2 halo)
NBB = 8  # b_out blocks: 7 full (4 wide) + 1 last (2 wide)
CW = [16, 16, 16, 14]  # c-chunk widths (sum 62); N = wc*31 <= 496
C0 = [0, 16, 32, 48]
CP = 18  # priority c-columns: chunk 0 reads c in [0, 16+2)

# Winograd F(2,3) weight transform G
G_MAT = np.array([[1, 0, 0], [0.5, 0.5, 0.5], [0.5, -0.5, 0.5], [0, 0, 1]], np.float64)

_CACHE = {}


def _build_weights(weight: np.ndarray, bias: np.ndarray):
    """Banded lhsT of G-transformed weights per (point p, c-tap k)."""
    # wt[p][co, ci, i, j, k] = sum_l G[p, l] * w[co, ci, i, j, k, l]
    wt = np.einsum("pl,ocijkl->pocijk", G_MAT, weight.astype(np.float64)).astype(
        np.float32
    )

    def banded(sa, n_ao, sb, n_bo):
        sa_sel = np.zeros((sa, n_ao, K), np.float32)
        for t in range(K):
            for o in range(n_ao):
                sa_sel[o + t, o, t] = 1.0
        sb_sel = np.zeros((sb, n_bo, K), np.float32)
        for t in range(K):
            for o in range(n_bo):
                sb_sel[o + t, o, t] = 1.0
        # lhsT[(db,ci,da), t=(p,k), (co,ao,bo)] — 12 taps side by side
        out = np.zeros((sb * CI * sa, 12, CO * n_ao * n_bo), np.float32)
        for p in range(4):
            for k in range(K):
                wpk = wt[p, :, :, :, :, k]  # (co, ci, i, j)
                m = np.einsum("dai,ebj,ocij->ecdoab", sa_sel, sb_sel, wpk)
                out[:, p * 3 + k, :] = m.reshape(sb * CI * sa, CO * n_ao * n_bo)
        return np.ascontiguousarray(out.reshape(sb * CI * sa, 12 * CO * n_ao * n_bo))

    w_main = banded(SA, 8, SB, 4)  # (120, 12*128)
    w_last = banded(SA, 8, 4, 2)  # (80, 12*64)
    bias_main = np.repeat(bias.astype(np.float32), 32).reshape(128, 1)
    bias_last = np.repeat(bias.astype(np.float32), 16).reshape(64, 1)
    return w_main, w_last, bias_main, bias_last


def _build_program():
    import concourse.bass as bass  # noqa: F401
    import concourse.mybir as mybir
    import concourse.tile as tile
    from concourse import bacc

    f32 = mybir.dt.float32
    bf16 = mybir.dt.bfloat16
    add = mybir.AluOpType.add
    sub = mybir.AluOpType.subtract

    nc = bacc.Bacc("TRN2", target_bir_lowering=False, debug=False, num_devices=8)
    # x with d deinterleaved: [ci, a, b, c, parity, 32]
    xs = nc.dram_tensor("x_slab", [CI, SA, B2, C, 2, 32], bf16, kind="ExternalInput")
    wm = nc.dram_tensor("w_main", [120, 12 * 128], bf16, kind="ExternalInput")
    wl = nc.dram_tensor("w_last", [80, 12 * 64], bf16, kind="ExternalInput")
    bm = nc.dram_tensor("bias_main", [128, 1], f32, kind="ExternalInput")
    bl = nc.dram_tensor("bias_last", [64, 1], f32, kind="ExternalInput")
    # partition-major blocks per (b-block, c-chunk, parity); host unscrambles
    out = nc.dram_tensor("out_blocks", [NBB, 4, 128, 2, 16 * TD], f32, kind="ExternalOutput")

    with tile.TileContext(nc) as tc:
        with (
            tc.tile_pool(name="w", bufs=1) as wpool,
            tc.tile_pool(name="rhs", bufs=3) as rpool,
            tc.tile_pool(name="xf", bufs=2) as xfpool,
            tc.tile_pool(name="psum", bufs=8, space="PSUM") as ppool,
            tc.tile_pool(name="tmp", bufs=3) as tpool,
            tc.tile_pool(name="ot", bufs=3) as opool,
        ):
            # weights ride the GpSimd DGE queue: off the critical rhs path
            w_main_t = wpool.tile([120, 12 * 128], bf16)
            nc.gpsimd.dma_start(w_main_t[:], wm[:])
            w_last_t = wpool.tile([80, 12 * 64], bf16)
            bias_main_t = wpool.tile([128, 1], f32)
            bias_last_t = wpool.tile([64, 1], f32)
            nc.gpsimd.dma_start(bias_main_t[:], bm[:])
            nc.gpsimd.dma_start(w_last_t[:], wl[:])
            nc.gpsimd.dma_start(bias_last_t[:], bl[:])

            for bb in range(NBB):
                b0 = bb * 4
                wb = SB if bb < NBB - 1 else 4  # b-window width
                wbo = 4 if bb < NBB - 1 else 2  # b_out width
                P = CI * SA * wb  # 120 or 80
                M = CO * 8 * wbo  # 128 or 64

                rhs_t = rpool.tile([P, C * D], bf16, tag="rhs")
                if bb == 0:
                    # priority: c<CP feeds chunk 0; split Sync+ACT queues
                    for db in range(wb):
                        q = nc.sync if db % 2 == 0 else nc.scalar
                        q.dma_start(
                            rhs_t[db * 20 : (db + 1) * 20, : CP * D],
                            xs[:, :, b0 + db, :CP].rearrange(
                                "ci a c two d -> (ci a) (c two d)"
                            ),
                        )
                    for db in range(wb):
                        q = nc.scalar if db % 2 == 0 else nc.sync
                        q.dma_start(
                            rhs_t[db * 20 : (db + 1) * 20, CP * D :],
                            xs[:, :, b0 + db, CP:].rearrange(
                                "ci a c two d -> (ci a) (c two d)"
                            ),
                        )
                else:
                    for db in range(wb):
                        nc.sync.dma_start(
                            rhs_t[db * 20 : (db + 1) * 20, :],
                            xs[:, :, b0 + db].rearrange(
                                "ci a c two d -> (ci a) (c two d)"
                            ),
                        )

                # input transforms: packed bf16 (DVE 3, Pool 1)
                xf_t = xfpool.tile([P, C * 4 * TD], bf16, tag="xf")
                x4 = rhs_t.rearrange("p (c two d) -> p c two d", two=2, d=32)
                xf4 = xf_t.rearrange("p (c q t) -> p c q t", q=4, t=TD)
                splits = [(0, CP), (CP, C)] if bb == 0 else [(0, C)]
                for lo, hi in splits:
                    s = slice(lo, hi)
                    xe0, xe1 = x4[:, s, 0, 0:31], x4[:, s, 0, 1:32]
                    xo0, xo1 = x4[:, s, 1, 0:31], x4[:, s, 1, 1:32]
                    nc.vector.tensor_sub(xf4[:, s, 0, :], xe0, xe1)  # p0
                    nc.vector.tensor_add(xf4[:, s, 1, :], xo0, xe1)  # p1
                    nc.vector.tensor_sub(xf4[:, s, 2, :], xe1, xo0)  # p2
                    nc.gpsimd.tensor_sub(xf4[:, s, 3, :], xo0, xo1)  # p3

                w_t = w_main_t if bb < NBB - 1 else w_last_t
                bias_t = bias_main_t if bb < NBB - 1 else bias_last_t

                for cc in range(4):
                    c0, wc = C0[cc], CW[cc]
                    N = wc * TD
                    ys = []
                    for p in range(4):
                        ps = ppool.tile([M, N], f32, tag="y")
                        for k in range(K):
                            nc.tensor.matmul(
                                ps.rearrange("m (c t) -> m c t", c=wc),
                                w_t[:, (p * 3 + k) * M : (p * 3 + k + 1) * M],
                                xf4[:, c0 + k : c0 + k + wc, p, :],
                                start=(k == 0),
                                stop=(k == 2),
                            )
                        ys.append(ps)
                    y0, y1, y2, y3 = ys
                    # inverse transform + bias; each Y read once from PSUM
                    s1 = tpool.tile([M, N], f32, tag="s1")
                    nc.scalar.copy(s1[:], y1[:])
                    s2 = tpool.tile([M, N], f32, tag="s2")
                    nc.scalar.copy(s2[:], y2[:])
                    u = tpool.tile([M, N], f32, tag="u")
                    nc.vector.scalar_tensor_tensor(u[:], s1[:], bias_t[:], y0[:], add, add)
                    v = tpool.tile([M, N], f32, tag="v")
                    nc.vector.scalar_tensor_tensor(v[:], s1[:], bias_t[:], y3[:], add, sub)
                    ot = opool.tile([M, 2 * N], f32, tag="ot")
                    ot2 = ot.rearrange("m (two n) -> m two n", two=2)
                    nc.gpsimd.tensor_add(ot2[:, 0], u[:], s2[:])  # even d
                    nc.gpsimd.tensor_sub(ot2[:, 1], v[:], s2[:])  # odd d
                    nc.scalar.dma_start(out[bb, cc, :M, :, :N], ot2[:])
    nc.compile()
    return nc


def kernel(x: np.ndarray, weight: np.ndarray, bias: np.ndarray) -> np.ndarray:
    from concourse.bass_utils import run_bass_kernel_spmd

    if "nc" not in _CACHE:
        _CACHE["nc"] = _build_program()
    nc = _CACHE["nc"]

    w_main, w_last, bias_main, bias_last = _build_weights(weight, bias)
    x_bf = x.astype(BF16)
    w_main = w_main.astype(BF16)
    w_last = w_last.astype(BF16)

    in_maps = []
    for core in range(8):
        b, q = divmod(core, 4)
        a0 = A0[q]
        slab = x_bf[b, :, a0 : a0 + SA]  # (ci, 10, 32, 64, 64)
        # deinterleave d into (parity, 32)
        slab = slab.reshape(CI, SA, B2, C, 32, 2).transpose(0, 1, 2, 3, 5, 4)
        in_maps.append(
            {
                "x_slab": np.ascontiguousarray(slab),
                "w_main": w_main,
                "w_last": w_last,
                "bias_main": bias_main,
                "bias_last": bias_last,
            }
        )

    res = run_bass_kernel_spmd(nc, in_maps, core_ids=list(range(8)))
    _CACHE["last_result"] = res

    out = np.empty((B, CO, AO, BO, CL, DL), np.float32)
    for core in range(8):
        b, q = divmod(core, 4)
        slab = _unscramble(res.results[core]["out_blocks"])  # (4, 8, 30, 62, 62)
        if q < 3:
            out[b, :, 8 * q : 8 * q + 8] = slab
        else:
            out[b, :, 24:30] = slab[:, 2:8]
    return out


def _unscramble(blocks: np.ndarray) -> np.ndarray:
    """[NBB, 4, 128, 2, 16*31] blocks -> (4, 8, 30, 62, 62) slab."""
    slab = np.empty((CO, 8, BO, CL, DL), np.float32)
    for bb in range(NBB):
        wbo = 4 if bb < NBB - 1 else 2
        m = CO * 8 * wbo
        for cc in range(4):
            c0, wc = C0[cc], CW[cc]
            blk = blocks[bb, cc, :m, :, : wc * TD].reshape(CO, 8, wbo, 2, wc, TD)
            # (co, ao, bo, parity, c, td) -> interleave parity into d
            blk = blk.transpose(0, 1, 2, 4, 5, 3).reshape(CO, 8, wbo, wc, DL)
            slab[:, :, bb * 4 : bb * 4 + wbo, c0 : c0 + wc, :] = blk
    return slab


# revision 11
# speedup vs baseline: 1.1462x; 1.1462x over previous
"""Trainium2 Bass kernel for 4D valid convolution.

x (2,2,32,32,64,64) f32, weight (4,2,3,3,3,3) f32, bias (4,) f32
-> out (2,4,30,30,62,62) f32  (valid cross-correlation + bias)

Strategy: 8 cores = batch(2) x a-quadrant(4). Each core computes
out[b, :, a_sel, :, :, :] from slab x[b, :, a0:a0+10, :, :, :].

The d-dimension kernel taps use Winograd F(2,3): 4 transform points per
2 outputs replace the 3 direct taps, cutting TensorE columns 1.5x.
The host sends x with d deinterleaved into (even, odd) planes so every
transform op is a packed (stride-1) bf16 tensor_tensor:
       p0=xe[t]-xe[t+1]  p1=xo[t]+xe[t+1]  p2=xe[t+1]-xo[t]
       p3=xo[t]-xo[t+1]            (xe[t]=x[2t], xo[t]=x[2t+1])
TensorE per (a,b)-block and c-chunk: for each point p, 3 matmuls
(c-taps k) accumulate Y_p in its own PSUM bank:
       K (partitions) = (b-window=6, ci=2, a-window=10) = 120
       M (psum)       = (co=4, a_out=8, b_out=4) = 128
       N              = wc x 31 d-pairs <= 496 (1 PSUM bank)
       lhsT = host-prebuilt banded matrices of G-transformed weights.
Inverse transform (each op reads at most one PSUM operand; bias folded
into the Act chain heads; Pool never touches PSUM):
       Act:  e2=Y2+bias          o3=bias-Y3
       DVE:  even=(e2+Y1)+Y0     odd=(o3+Y1)-Y2   (two stt ops each)
Input transforms run on GpSimd only, emitted one b-block ahead so the
PE never waits on them.  Per-block engine budget: PE 9.6us, DVE ~8us,
Act ~8us (chain heads + store issue), Pool ~7us.

Startup: the first chunk's rhs columns (c<18) load first, split across
the Sync+ACT DGE queues; weights ride the GpSimd queue. Output goes to
DRAM partition-major per (b-block, c-chunk, parity); host unscrambles.
"""

import sys

if "/opt/trn_rl_repo" not in sys.path:
    sys.path.insert(0, "/opt/trn_rl_repo")

import ml_dtypes
import numpy as np

BF16 = ml_dtypes.bfloat16

B, CI, CO = 2, 2, 4
A, B2, C, D = 32, 32, 64, 64
AO, BO, CL, DL = 30, 30, 62, 62
K = 3
TD = 31  # d-pair tiles (Winograd F(2,3) stride 2)

# per-core a-slab starts; each core computes 8 output a-rows (q=3 overlaps q=2)
A0 = [0, 8, 16, 22]
SA = 10  # a-window (8 outputs + 2 halo)
SB = 6  # b-window per block (4 outputs + 2 halo)
NBB = 8  # b_out blocks: 7 full (4 wide) + 1 last (2 wide)
CW = [16, 16, 16, 14]  # c-chunk widths (sum 62); N = wc*31 <= 496
C0 = [0, 16, 32, 48]
CP = 18  # priority c-columns: chunk 0 reads c in [0, 16+2)

# Winograd F(2,3) weight transform G
G_MAT = np.array([[1, 0, 0], [0.5, 0.5, 0.5], [0.5, -0.5, 0.5], [0, 0, 1]], np.float64)

_CACHE = {}


def _build_weights(weight: np.ndarray, bias: np.ndarray):
    """Banded lhsT of G-transformed weights per (point p, c-tap k)."""
    # wt[p][co, ci, i, j, k] = sum_l G[p, l] * w[co, ci, i, j, k, l]
    wt = np.einsum("pl,ocijkl->pocijk", G_MAT, weight.astype(np.float64)).astype(
        np.float32
    )

    def banded(sa, n_ao, sb, n_bo):
        sa_sel = np.zeros((sa, n_ao, K), np.float32)
        for t in range(K):
            for o in range(n_ao):
                sa_sel[o + t, o, t] = 1.0
        sb_sel = np.zeros((sb, n_bo, K), np.float32)
        for t in range(K):
            for o in range(n_bo):
                sb_sel[o + t, o, t] = 1.0
        # lhsT[(db,ci,da), t=(p,k), (co,ao,bo)] — 12 taps side by side
        out = np.zeros((sb * CI * sa, 12, CO * n_ao * n_bo), np.float32)
        for p in range(4):
            for k in range(K):
                wpk = wt[p, :, :, :, :, k]  # (co, ci, i, j)
                m = np.einsum("dai,ebj,ocij->ecdoab", sa_sel, sb_sel, wpk)
                out[:, p * 3 + k, :] = m.reshape(sb * CI * sa, CO * n_ao * n_bo)
        return np.ascontiguousarray(out.reshape(sb * CI * sa, 12 * CO * n_ao * n_bo))

    w_main = banded(SA, 8, SB, 4)  # (120, 12*128)
    w_last = banded(SA, 8, 4, 2)  # (80, 12*64)
    bias_main = np.repeat(bias.astype(np.float32), 32).reshape(128, 1)
    bias_last = np.repeat(bias.astype(np.float32), 16).reshape(64, 1)
    return w_main, w_last, bias_main, bias_last


def _build_program():
    import concourse.bass as bass  # noqa: F401
    import concourse.mybir as mybir
    import concourse.tile as tile
    from concourse import bacc

    f32 = mybir.dt.float32
    bf16 = mybir.dt.bfloat16
    add = mybir.AluOpType.add
    sub = mybir.AluOpType.subtract
    mult = mybir.AluOpType.mult
    ident = mybir.ActivationFunctionType.Identity

    nc = bacc.Bacc("TRN2", target_bir_lowering=False, debug=False, num_devices=8)
    # x with d deinterleaved: [ci, a, b, c, parity, 32]
    xs = nc.dram_tensor("x_slab", [CI, SA, B2, C, 2, 32], bf16, kind="ExternalInput")
    wm = nc.dram_tensor("w_main", [120, 12 * 128], bf16, kind="ExternalInput")
    wl = nc.dram_tensor("w_last", [80, 12 * 64], bf16, kind="ExternalInput")
    bm = nc.dram_tensor("bias_main", [128, 1], f32, kind="ExternalInput")
    bl = nc.dram_tensor("bias_last", [64, 1], f32, kind="ExternalInput")
    # partition-major blocks per (b-block, c-chunk, parity); host unscrambles
    out = nc.dram_tensor("out_blocks", [NBB, 4, 128, 2, 16 * TD], f32, kind="ExternalOutput")

    with tile.TileContext(nc) as tc:
        with (
            tc.tile_pool(name="w", bufs=1) as wpool,
            tc.tile_pool(name="rhs", bufs=3) as rpool,
            tc.tile_pool(name="xf", bufs=2) as xfpool,
            tc.tile_pool(name="psum", bufs=8, space="PSUM") as ppool,
            tc.tile_pool(name="tmp", bufs=3) as tpool,
            tc.tile_pool(name="ot", bufs=3) as opool,
        ):
            # weights ride the GpSimd DGE queue: off the critical rhs path
            w_main_t = wpool.tile([120, 12 * 128], bf16)
            nc.gpsimd.dma_start(w_main_t[:], wm[:])
            w_last_t = wpool.tile([80, 12 * 64], bf16)
            bias_main_t = wpool.tile([128, 1], f32)
            bias_last_t = wpool.tile([64, 1], f32)
            nc.gpsimd.dma_start(bias_main_t[:], bm[:])
            nc.gpsimd.dma_start(w_last_t[:], wl[:])
            nc.gpsimd.dma_start(bias_last_t[:], bl[:])

            def bb_dims(bb):
                wb = SB if bb < NBB - 1 else 4  # b-window width
                wbo = 4 if bb < NBB - 1 else 2  # b_out width
                return wb, CI * SA * wb, CO * 8 * wbo  # wb, P, M

            # Phase A: all rhs loads (run ahead, bounded by rpool bufs)
            rhs_tiles = []
            for bb in range(NBB):
                b0 = bb * 4
                wb, P, M = bb_dims(bb)
                rhs_t = rpool.tile([P, C * D], bf16, tag="rhs")
                rhs_tiles.append(rhs_t)
                if bb == 0:
                    # priority: c<CP feeds chunk 0; split Sync+ACT queues
                    for db in range(wb):
                        q = nc.sync if db % 2 == 0 else nc.scalar
                        q.dma_start(
                            rhs_t[db * 20 : (db + 1) * 20, : CP * D],
                            xs[:, :, b0 + db, :CP].rearrange(
                                "ci a c two d -> (ci a) (c two d)"
                            ),
                        )
                    for db in range(wb):
                        q = nc.scalar if db % 2 == 0 else nc.sync
                        q.dma_start(
                            rhs_t[db * 20 : (db + 1) * 20, CP * D :],
                            xs[:, :, b0 + db, CP:].rearrange(
                                "ci a c two d -> (ci a) (c two d)"
                            ),
                        )
                else:
                    for db in range(wb):
                        nc.sync.dma_start(
                            rhs_t[db * 20 : (db + 1) * 20, :],
                            xs[:, :, b0 + db].rearrange(
                                "ci a c two d -> (ci a) (c two d)"
                            ),
                        )

            xf_tiles = {}

            def emit_transform(bb, splits):
                # input transforms: packed bf16 tensor_tensor on GpSimd only
                wb, P, M = bb_dims(bb)
                if bb not in xf_tiles:
                    xf_tiles[bb] = xfpool.tile([P, C * 4 * TD], bf16, tag="xf", name=f"xf{bb}")
                xf4 = xf_tiles[bb].rearrange("p (c q t) -> p c q t", q=4, t=TD)
                x4 = rhs_tiles[bb].rearrange("p (c two d) -> p c two d", two=2, d=32)
                for lo, hi in splits:
                    s = slice(lo, hi)
                    xe0, xe1 = x4[:, s, 0, 0:31], x4[:, s, 0, 1:32]
                    xo0, xo1 = x4[:, s, 1, 0:31], x4[:, s, 1, 1:32]
                    nc.gpsimd.tensor_sub(xf4[:, s, 0, :], xe0, xe1)  # p0
                    nc.gpsimd.tensor_add(xf4[:, s, 1, :], xo0, xe1)  # p1
                    nc.gpsimd.tensor_sub(xf4[:, s, 2, :], xe1, xo0)  # p2
                    nc.gpsimd.tensor_sub(xf4[:, s, 3, :], xo0, xo1)  # p3

            emit_transform(0, [(0, CP), (CP, C)])

            for bb in range(NBB):
                wb, P, M = bb_dims(bb)
                xf4 = xf_tiles[bb].rearrange("p (c q t) -> p c q t", q=4, t=TD)
                w_t = w_main_t if bb < NBB - 1 else w_last_t
                bias_t = bias_main_t if bb < NBB - 1 else bias_last_t

                for cc in range(4):
                    if cc == 2 and bb + 1 < NBB:
                        emit_transform(bb + 1, [(0, C)])  # one block ahead
                    c0, wc = C0[cc], CW[cc]
                    N = wc * TD
                    ys = []
                    for p in range(4):
                        ps = ppool.tile([M, N], f32, tag="y")
                        for k in range(K):
                            nc.tensor.matmul(
                                ps.rearrange("m (c t) -> m c t", c=wc),
                                w_t[:, (p * 3 + k) * M : (p * 3 + k + 1) * M],
                                xf4[:, c0 + k : c0 + k + wc, p, :],
                                start=(k == 0),
                                stop=(k == 2),
                            )
                        ys.append(ps)
                    y0, y1, y2, y3 = ys
                    ot = opool.tile([M, 2 * N], f32, tag="ot")
                    ot2 = ot.rearrange("m (two n) -> m two n", two=2)
                    # inverse transform; <=1 PSUM operand per op, bias folded
                    e2 = tpool.tile([M, N], f32, tag="e2")
                    nc.scalar.activation(e2[:], y2[:], ident, bias=bias_t[:], scale=1.0)
                    o3 = tpool.tile([M, N], f32, tag="o3")
                    nc.scalar.activation(o3[:], y3[:], ident, bias=bias_t[:], scale=-1.0)
                    u = tpool.tile([M, N], f32, tag="u")
                    nc.vector.scalar_tensor_tensor(u[:], e2[:], 1.0, y1[:], mult, add)
                    nc.vector.scalar_tensor_tensor(ot2[:, 0], u[:], 1.0, y0[:], mult, add)
                    v = tpool.tile([M, N], f32, tag="v")
                    nc.vector.scalar_tensor_tensor(v[:], o3[:], 1.0, y1[:], mult, add)
                    nc.vector.scalar_tensor_tensor(ot2[:, 1], v[:], 1.0, y2[:], mult, sub)
                    nc.scalar.dma_start(out[bb, cc, :M, :, :N], ot2[:])
    nc.compile()
    return nc


def kernel(x: np.ndarray, weight: np.ndarray, bias: np.ndarray) -> np.ndarray:
    from concourse.bass_utils import run_bass_kernel_spmd

    if "nc" not in _CACHE:
        _CACHE["nc"] = _build_program()
    nc = _CACHE["nc"]

    w_main, w_last, bias_main, bias_last = _build_weights(weight, bias)
    x_bf = x.astype(BF16)
    w_main = w_main.astype(BF16)
    w_last = w_last.astype(BF16)

    in_maps = []
    for core in range(8):
        b, q = divmod(core, 4)
        a0 = A0[q]
        slab = x_bf[b, :, a0 : a0 + SA]  # (ci, 10, 32, 64, 64)
        # deinterleave d into (parity, 32)
        slab = slab.reshape(CI, SA, B2, C, 32, 2).transpose(0, 1, 2, 3, 5, 4)
        in_maps.append(
            {
                "x_slab": np.ascontiguousarray(slab),
                "w_main": w_main,
                "w_last": w_last,
                "bias_main": bias_main,
                "bias_last": bias_last,
            }
        )

    res = run_bass_kernel_spmd(nc, in_maps, core_ids=list(range(8)))
    _CACHE["last_result"] = res

    out = np.empty((B, CO, AO, BO, CL, DL), np.float32)
    for core in range(8):
        b, q = divmod(core, 4)
        slab = _unscramble(res.results[core]["out_blocks"])  # (4, 8, 30, 62, 62)
        if q < 3:
            out[b, :, 8 * q : 8 * q + 8] = slab
        else:
            out[b, :, 24:30] = slab[:, 2:8]
    return out


def _unscramble(blocks: np.ndarray) -> np.ndarray:
    """[NBB, 4, 128, 2, 16*31] blocks -> (4, 8, 30, 62, 62) slab."""
    slab = np.empty((CO, 8, BO, CL, DL), np.float32)
    for bb in range(NBB):
        wbo = 4 if bb < NBB - 1 else 2
        m = CO * 8 * wbo
        for cc in range(4):
            c0, wc = C0[cc], CW[cc]
            blk = blocks[bb, cc, :m, :, : wc * TD].reshape(CO, 8, wbo, 2, wc, TD)
            # (co, ao, bo, parity, c, td) -> interleave parity into d
            blk = blk.transpose(0, 1, 2, 4, 5, 3).reshape(CO, 8, wbo, wc, DL)
            slab[:, :, bb * 4 : bb * 4 + wbo, c0 : c0 + wc, :] = blk
    return slab


# revision 12
# speedup vs baseline: 1.4092x; 1.2295x over previous
"""Trainium2 Bass kernel for 4D valid convolution.

x (2,2,32,32,64,64) f32, weight (4,2,3,3,3,3) f32, bias (4,) f32
-> out (2,4,30,30,62,62) f32  (valid cross-correlation + bias)

Strategy: 8 cores = batch(2) x a-quadrant(4). Each core computes
out[b, :, a_sel, :, :, :] from slab x[b, :, a0:a0+10, :, :, :].

TensorE mapping per core (bf16 inputs, f32 PSUM accumulate):
  K (contraction, partitions) = (b-window=6, ci=2, a-window=10) = 120
  M (psum partitions)         = (co=4, a_out=8, b_out=4) = 128
  N (streamed free dim)       = contiguous (c,d) output pixels, <=496
Host prebuilds banded lhsT matrices (one per (k,l) tap, side by side in
one [120, 9*128] array -> a single DMA); the 9 (k,l) taps accumulate in
PSUM using (c,d)-shifted views of the same SBUF x tile, so each weight
load serves a full 496-column stream and the PE runs back-to-back at
~N cycles/matmul. Loads issue from the Sync DGE queue, stores from the
ACT queue (keeps Sync free to prefetch), evictions (bias add) on DVE.
Output goes to DRAM partition-major per (b-block, c-chunk); the host
unscrambles (SBUF-side multi-dim partition DMAs mislower, so the device
only ever does flat [P, N] stores).

Measured: ~145 us HW exec (8 cores), max rel err ~2.2e-3 vs f32
reference (bf16 input rounding; PE pitch ~210 ns/matmul = bf16
streaming roofline for this shape).
"""

import sys

if "/opt/trn_rl_repo" not in sys.path:
    sys.path.insert(0, "/opt/trn_rl_repo")

import ml_dtypes
import numpy as np

BF16 = ml_dtypes.bfloat16

B, CI, CO = 2, 2, 4
A, B2, C, D = 32, 32, 64, 64
AO, BO, CL, DL = 30, 30, 62, 62
K = 3

# per-core a-slab starts; each core computes 8 output a-rows (q=3 overlaps q=2)
A0 = [0, 8, 16, 22]
SA = 10  # a-window (8 outputs + 2 halo)
SB = 6  # b-window per block (4 outputs + 2 halo)
NBB = 8  # b_out blocks: 7 full (4 wide) + 1 last (2 wide)
NCC = 8  # c chunks: 7 full (8 wide) + 1 last (6 wide)

_CACHE = {}


def _build_weights(weight: np.ndarray, bias: np.ndarray):
    """Banded lhsT matrices per (k,l) tap, plus per-partition bias vectors."""
    w = weight.astype(np.float32)

    def banded(sa, n_ao, sb, n_bo):
        # sel[d, o, t] = 1 if d == o + t
        sa_sel = np.zeros((sa, n_ao, K), np.float32)
        for t in range(K):
            for o in range(n_ao):
                sa_sel[o + t, o, t] = 1.0
        sb_sel = np.zeros((sb, n_bo, K), np.float32)
        for t in range(K):
            for o in range(n_bo):
                sb_sel[o + t, o, t] = 1.0
        # lhsT[(db,ci,da), t=(k,l), (co,ao,bo)] — taps side by side in columns
        # so the whole thing loads with a single 2D DMA into [P, 9*M]
        out = np.zeros((sb * CI * sa, 9, CO * n_ao * n_bo), np.float32)
        for k in range(K):
            for l in range(K):
                wkl = w[:, :, :, :, k, l]  # (co, ci, i, j)
                m = np.einsum("dai,ebj,ocij->ecdoab", sa_sel, sb_sel, wkl)
                out[:, k * 3 + l, :] = m.reshape(sb * CI * sa, CO * n_ao * n_bo)
        return np.ascontiguousarray(out.reshape(sb * CI * sa, 9 * CO * n_ao * n_bo))

    w_main = banded(SA, 8, SB, 4)  # (9, 120, 128)
    w_last = banded(SA, 8, 4, 2)  # (9, 80, 64)
    bias_main = np.repeat(bias.astype(np.float32), 32).reshape(128, 1)
    bias_last = np.repeat(bias.astype(np.float32), 16).reshape(64, 1)
    return w_main, w_last, bias_main, bias_last


def _build_program():
    import concourse.bass as bass  # noqa: F401
    import concourse.mybir as mybir
    import concourse.tile as tile
    from concourse import bacc

    f32 = mybir.dt.float32
    bf16 = mybir.dt.bfloat16

    nc = bacc.Bacc("TRN2", target_bir_lowering=False, debug=False, num_devices=8)
    xs = nc.dram_tensor("x_slab", [CI, SA, B2, C, D], bf16, kind="ExternalInput")
    wm = nc.dram_tensor("w_main", [120, 9 * 128], bf16, kind="ExternalInput")
    wl = nc.dram_tensor("w_last", [80, 9 * 64], bf16, kind="ExternalInput")
    bm = nc.dram_tensor("bias_main", [128, 1], f32, kind="ExternalInput")
    bl = nc.dram_tensor("bias_last", [64, 1], f32, kind="ExternalInput")
    # partition-major blocks: [bb, cc, m, n]; host unscrambles (cheap numpy)
    out = nc.dram_tensor(
        "out_blocks", [NBB, NCC, 128, 8 * DL], f32, kind="ExternalOutput"
    )

    with tile.TileContext(nc) as tc:
        with (
            tc.tile_pool(name="w", bufs=1) as wpool,
            tc.tile_pool(name="rhs", bufs=8) as rpool,
            tc.tile_pool(name="psum", bufs=8, space="PSUM") as ppool,
            tc.tile_pool(name="ot", bufs=4) as opool,
        ):
            # weights ride the GpSimd DGE queue: off the critical rhs path
            w_main_t = wpool.tile([120, 9 * 128], bf16)
            nc.gpsimd.dma_start(w_main_t[:], wm[:])
            w_last_t = wpool.tile([80, 9 * 64], bf16)
            bias_main_t = wpool.tile([128, 1], f32)
            bias_last_t = wpool.tile([64, 1], f32)
            nc.gpsimd.dma_start(bias_main_t[:], bm[:])
            nc.gpsimd.dma_start(w_last_t[:], wl[:])
            nc.gpsimd.dma_start(bias_last_t[:], bl[:])

            CP = 10  # priority c-cols: chunk 0 reads c in [0, 8+2)

            for bb in range(NBB):
                b0 = bb * 4
                wb = SB if bb < NBB - 1 else 4  # b-window width
                wbo = 4 if bb < NBB - 1 else 2  # b_out width
                P = CI * SA * wb  # 120 or 80
                M = CO * 8 * wbo  # 128 or 64

                rhs_t = rpool.tile([P, C * D], bf16, tag="rhs")
                if bb == 0:
                    # priority: c<CP feeds chunk 0; split Sync+ACT queues so
                    # the PE starts ~2us after the DGE queues come up
                    for db in range(wb):
                        q = nc.sync if db % 2 == 0 else nc.scalar
                        q.dma_start(
                            rhs_t[db * 20 : (db + 1) * 20, : CP * D],
                            xs[:, :, b0 + db, :CP].rearrange("ci a c d -> (ci a) (c d)"),
                        )
                    for db in range(wb):
                        q = nc.scalar if db % 2 == 0 else nc.sync
                        q.dma_start(
                            rhs_t[db * 20 : (db + 1) * 20, CP * D :],
                            xs[:, :, b0 + db, CP:].rearrange("ci a c d -> (ci a) (c d)"),
                        )
                else:
                    for db in range(wb):
                        nc.sync.dma_start(
                            rhs_t[db * 20 : (db + 1) * 20, :],
                            xs[:, :, b0 + db].rearrange("ci a c d -> (ci a) (c d)"),
                        )
                rhs3 = rhs_t.rearrange("p (c d) -> p c d", c=C)
                w_t = w_main_t if bb < NBB - 1 else w_last_t
                bias_t = bias_main_t if bb < NBB - 1 else bias_last_t

                for cc in range(NCC):
                    c0 = cc * 8
                    wc = 8 if cc < NCC - 1 else 6
                    N = wc * DL
                    ps = ppool.tile([M, N], f32, tag="ps")
                    for t in range(9):
                        k, l = divmod(t, 3)
                        rv = rhs3[:, c0 + k : c0 + k + wc, l : l + DL]
                        nc.tensor.matmul(
                            ps.rearrange("m (c d) -> m c d", c=wc),
                            w_t[:, t * M : (t + 1) * M],
                            rv,
                            start=(t == 0),
                            stop=(t == 8),
                        )
                    ot = opool.tile([M, N], f32, tag="ot")
                    nc.vector.tensor_scalar_add(ot[:], ps[:], bias_t[:])
                    # store from the ACT queue: Sync stays free for loads
                    nc.scalar.dma_start(out[bb, cc, :M, :N], ot[:])
    nc.compile()
    return nc


def kernel(x: np.ndarray, weight: np.ndarray, bias: np.ndarray) -> np.ndarray:
    from concourse.bass_utils import run_bass_kernel_spmd

    if "nc" not in _CACHE:
        _CACHE["nc"] = _build_program()
    nc = _CACHE["nc"]

    w_main, w_last, bias_main, bias_last = _build_weights(weight, bias)
    x_bf = x.astype(BF16)
    w_main = w_main.astype(BF16)
    w_last = w_last.astype(BF16)

    in_maps = []
    for core in range(8):
        b, q = divmod(core, 4)
        a0 = A0[q]
        in_maps.append(
            {
                "x_slab": np.ascontiguousarray(x_bf[b, :, a0 : a0 + SA]),
                "w_main": w_main,
                "w_last": w_last,
                "bias_main": bias_main,
                "bias_last": bias_last,
            }
        )

    res = run_bass_kernel_spmd(nc, in_maps, core_ids=list(range(8)))
    _CACHE["last_result"] = res

    out = np.empty((B, CO, AO, BO, CL, DL), np.float32)
    for core in range(8):
        b, q = divmod(core, 4)
        slab = _unscramble(res.results[core]["out_blocks"])  # (4, 8, 30, 62, 62)
        if q < 3:
            out[b, :, 8 * q : 8 * q + 8] = slab
        else:
            out[b, :, 24:30] = slab[:, 2:8]
    return out


def _unscramble(blocks: np.ndarray) -> np.ndarray:
    """[NBB, NCC, 128, 8*62] partition-major blocks -> (4, 8, 30, 62, 62) slab."""
    slab = np.empty((CO, 8, BO, CL, DL), np.float32)
    for bb in range(NBB):
        wbo = 4 if bb < NBB - 1 else 2
        m = CO * 8 * wbo
        for cc in range(NCC):
            wc = 8 if cc < NCC - 1 else 6
            n = wc * DL
            blk = blocks[bb, cc, :m, :n].reshape(CO, 8, wbo, wc, DL)
            slab[:, :, bb * 4 : bb * 4 + wbo, cc * 8 : cc * 8 + wc, :] = blk
    return slab

